# revision 25
# baseline (speedup 1.0000x reference)
"""GCNConv (transform + symmetric-norm aggregate + sigmoid) on 8 Trainium2 NeuronCores.

out_i = sigmoid(dinv_i * sum_{j->i} dinv_j*(xW)_j + dinv_i^2*(xW)_i + b),
dinv = 1/sqrt(1 + in_degree).

Device algorithm (SPMD over 8 cores; per-core differences are pure data):
  pass A: g = (dinv*x) @ W in bf16 for all nodes on every core (dinv folded
          into x on the host; tiled bf16 matmul from a host-transposed x;
          g stored to HBM as contiguous 128B rows — 64 bf16 payload, no
          pad — this core's own rows also kept in SBUF)
  pass B: per pair of 128-dst-node tiles: 2 dma_gather calls (one per
          partition-half; the two tile-parity sub-tables of a half share
          an in_ap and adjacent idx columns, so one call covers both —
          halves the 994ns/call SWDGE fixed overhead; calls rotate over
          the 4 SWDGE queues) pull g[src] rows for the pair's
          dst-bucketed edge lists. Gather payloads and strides
          must be 256B multiples, so each descriptor fetches a PAIR of g
          rows (elem_size=elem_step=128 bf16) and the wanted 64-elem half
          is picked when slicing the matmul rhs: sub-table q = (src
          partition high bit, src tile parity), so all slots of a chunk
          share the same half; idx = (p%64)*(nt_pad/2) + t_rot/2 fits
          int16.
          Per tile a one-hot S is built in bf16 on the DVE (is_equal of
          slot dst-ids vs an iota row, operands carrying a packed stride-1
          16-bit inner pair to hit the 2x DVE mode); segment-sum via bf16
          PE matmuls accumulated in f32 PSUM (self-loop chunk reads the
          SBUF own rows); finalize per 7-tile group: when b==0, one Act
          sigmoid per tile with the per-partition dinv_dst scale reading
          PSUM directly (general path: DVE mult+add then sigmoid), then
          one batched store.

Earlier HW A/B results baked in here: 4 SWDGE queues ~3.5x faster than 1
(gathers serialize per queue); per-core-count gather truncation via
reg_load + trailing -1 idxs works but is net-negative on HW (flags kept,
off); merged 2-tile calls measured >= 1-tile single-packet calls.

Each core's inputs are rotated by its tile offset so the program is address-
uniform: core c sees global node-tile (t + c*nt_core) % nt_pad at position t,
and its own output tiles are always tiles [0, nt_core).

Host side only re-formats data: edge bucket sort by (dst tile, src
sub-table) with in-bucket ordering by gather row for HBM locality,
degree/dinv, dinv*x fold, padding, int16 index encoding, per-core rotation.
"""

import sys

for _p in ("/opt/trn_rl_repo", "/root/.axon_site/_ro/trn_rl_repo"):
    if _p not in sys.path:
        sys.path.append(_p)

import ml_dtypes
import numpy as np

import concourse.bacc as bacc
import concourse.bass as bass
import concourse.mybir as mybir
import concourse.tile as tile
from concourse.bass import ts
from concourse.bass_utils import run_bass_kernel_spmd

P = 128
N_CORES = 8
BATCH_A = 16  # node tiles per pass-A iteration
FIN_B = 7  # dst tiles per pass-B finalize group
NQ = 4  # sub-tables: (src partition high bit, src tile parity)
NQUEUES = 4  # SWDGE queues; sub-table q -> queue q
GROW = 64  # g-table row width in bf16 elems (128B rows, all payload)
GSTEP = 128  # gather elem_step in elems (256B = 2 g rows per descriptor stride)
PAD_NEG1 = False  # gather padding slots use idx -1 (only valid with TRUNC)
TRUNC = False  # truncate gathers via per-core count registers (net-negative on HW)
PAIR = 2  # dst tiles merged per dma_gather call (1 = one call per tile)
GATH_BUFS = 5  # gather pool depth

BF16 = ml_dtypes.bfloat16

_prog_cache: dict = {}


def _plan(n_nodes: int):
    nt_real = -(-n_nodes // P)
    nt_pad = nt_real
    while (
        nt_pad % N_CORES
        or (nt_pad // N_CORES) % FIN_B
        or (nt_pad // N_CORES) % PAIR
        or (nt_pad // N_CORES) % 2  # parity sub-tables need even rotation
        or nt_pad % BATCH_A
    ):
        nt_pad += 1
    return nt_real, nt_pad, nt_pad * P, nt_pad // N_CORES


def preprocess(x: np.ndarray, edge_index: np.ndarray, W: np.ndarray, b: np.ndarray):
    n_nodes, hid = x.shape
    out_dim = W.shape[1]
    nt_real, nt_pad, npad, nt_core = _plan(n_nodes)

    src = np.ascontiguousarray(edge_index[0]).astype(np.int64)
    dst = np.ascontiguousarray(edge_index[1]).astype(np.int64)
    e = src.shape[0]

    deg = np.bincount(dst, minlength=npad).astype(np.float64) + 1.0  # self-loop
    dinv_full = (1.0 / np.sqrt(deg)).astype(np.float32)  # [npad]

    # bucket edges by (dst tile, src sub-table); within a bucket sort by
    # gather row index for HBM locality during the random-access gathers
    tile_of = dst // P
    q_of = ((src % P) // 64) * 2 + ((src // P) % 2)
    grp = tile_of * NQ + q_of
    subkey = ((src % P) % 64) * nt_pad + (src // P)
    order = np.lexsort((subkey, grp))
    src_s = src[order]
    dst_s = dst[order]
    grp_s = grp[order]

    grp_counts = np.bincount(grp_s, minlength=nt_pad * NQ)
    jq = int(max(1, -(-int(grp_counts.max()) // P)))  # chunks per (tile, quarter)
    jc = NQ * jq + 1  # chunks per tile incl. own/self-loop chunk
    slot_cap = jq * P

    grp_start = np.zeros(nt_pad * NQ, dtype=np.int64)
    np.cumsum(grp_counts[:-1], out=grp_start[1:])
    pos = np.arange(e, dtype=np.int64) - grp_start[grp_s]
    slot = grp_s * slot_cap + pos

    # per-edge gather info (tile-rotation applied per core later)
    nslots = nt_pad * NQ * slot_cap
    loc_pp = np.zeros(nslots, dtype=np.int64)
    loc_tg = np.zeros(nslots, dtype=np.int64)
    valid = np.zeros(nslots, dtype=bool)
    dl_flat = np.full(nslots, -1.0, dtype=np.float32)
    loc_pp[slot] = (src_s % P) % 64
    loc_tg[slot] = src_s // P
    valid[slot] = True
    dl_flat[slot] = (dst_s - (dst_s // P) * P).astype(np.float32)

    loc_pp3 = loc_pp.reshape(nt_pad, NQ, slot_cap)
    loc_tg3 = loc_tg.reshape(nt_pad, NQ, slot_cap)
    valid3 = valid.reshape(nt_pad, NQ, slot_cap)

    # dl input [P, nt_pad, jc, 2]: chunk cc=(qr*jq+j) at col t*jc+cc; own chunk
    # last; each value duplicated in an inner pair so the DVE one-hot build
    # reads a packed stride-1 16-bit pair (enables the 2x DVE perf mode)
    dl4 = dl_flat.reshape(nt_pad, NQ * jq, P)  # [t, cc, p]
    dl_all = np.empty((P, nt_pad, jc, 2), dtype=np.float32)
    dl_all[:, :, : NQ * jq, 0] = dl4.transpose(2, 0, 1)
    dl_all[:, :, NQ * jq, 0] = np.arange(P, dtype=np.float32)[:, None]
    dl_all[:, :, :, 1] = dl_all[:, :, :, 0]

    # per-(tile, quarter) gather valid-idx counts (exact; ucode contract:
    # num_idxs_reg == count of non-negative idxs, padding idxs = -1)
    gcnt_all = np.maximum(grp_counts.reshape(nt_pad, NQ), 1).astype(np.int32)
    # groups with zero edges keep one valid dummy idx (row 0, dl=-1 masks it)
    valid3[:, :, 0] |= grp_counts.reshape(nt_pad, NQ) == 0

    # dinv folded into x rows; both also shipped for the dst-side scale
    xs = np.asarray(x, np.float32) * dinv_full[:n_nodes, None]
    xT = np.zeros((hid, npad), dtype=BF16)
    xT[:, :n_nodes] = xs.T.astype(BF16)
    dinv2d = dinv_full.reshape(nt_pad, P).T.copy()  # [P, nt_pad]

    b_bcast = np.broadcast_to(np.asarray(b, np.float32), (P, out_dim)).copy()
    bz = bool(np.all(np.asarray(b) == 0.0))

    n_call = PAIR * slot_cap  # idxs per dma_gather call (PAIR tiles x quarter)
    cols_call = n_call // 16
    npairs = nt_core // PAIR

    shared = dict(W=np.asarray(W, np.float32).astype(BF16), b_bcast=b_bcast)
    per_core = []
    for c in range(N_CORES):
        t0 = c * nt_core
        xr = np.roll(xT, -t0 * P, axis=1)
        dvc = np.ascontiguousarray(np.roll(dinv2d, -t0, axis=1)[:, :nt_core])
        dlc = np.ascontiguousarray(
            dl_all[:, t0 : t0 + nt_core].reshape(P, nt_core * jc * 2).astype(BF16)
        )
        # per-call valid-idx count: non-last pair members are dummy-filled to
        # slot_cap (interior -1 is illegal), last member keeps its exact count
        cntp = gcnt_all[t0 : t0 + nt_core].reshape(npairs, PAIR, NQ)
        gcnt = (
            (slot_cap * (PAIR - 1) + cntp[:, -1, :])
            .astype(np.int32)
            .reshape(1, npairs * NQ)
        )
        # int16 gather locals with rotated tile index; padding slots = -1.
        # Rotation by t0 (even: nt_core is even) preserves tile parity, so
        # the per-edge sub-table (chosen pre-rotation) stays correct and
        # idx = (p%64)*(nt_pad/2) + t_rot/2 addresses 256B row-pairs.
        tg_rot = (loc_tg3[t0 : t0 + nt_core] - t0) % nt_pad  # [nt_core, NQ, slot_cap]
        loc = np.where(
            valid3[t0 : t0 + nt_core] if PAD_NEG1 else True,
            loc_pp3[t0 : t0 + nt_core] * (nt_pad // 2) + tg_rot // 2,
            -1,
        ).astype(np.int16)
        # merge PAIR consecutive tiles per call; non-last members' -1 padding
        # becomes dummy idx 0 (valid) so negatives stay strictly trailing
        locp = np.ascontiguousarray(
            loc.reshape(npairs, PAIR, NQ, slot_cap).transpose(0, 2, 1, 3)
        )
        if PAIR > 1:
            head = locp[:, :, :-1, :]
            head[head < 0] = 0
        # per call: idx i -> [i%16, i//16]; stack calls on cols; replicate x8
        loc_b = locp.reshape(npairs * NQ, cols_call, 16).transpose(0, 2, 1)
        idx16 = np.tile(
            loc_b.transpose(1, 0, 2).reshape(16, npairs * NQ * cols_call), (8, 1)
        )
        per_core.append(
            dict(
                xT=xr,
                dinv=dvc,
                dl=dlc,
                gcnt=gcnt,
                idx16=np.ascontiguousarray(idx16),
            )
        )
    meta = dict(
        n_nodes=n_nodes,
        hid=hid,
        out_dim=out_dim,
        nt_pad=nt_pad,
        npad=npad,
        nt_core=nt_core,
        jq=jq,
        jc=jc,
        bz=bz,
    )
    return meta, shared, per_core


def build_program(meta, variant="full"):
    reps = 1
    loopn = 0
    if "loop" in variant:  # e.g. "fullloop16": body once, inside a HW loop
        variant, _, r = variant.partition("loop")
        loopn = int(r)
    elif "x" in variant:
        variant, _, r = variant.partition("x")
        reps = int(r)
    hid, out_dim = meta["hid"], meta["out_dim"]
    nt_pad, nt_core = meta["nt_pad"], meta["nt_core"]
    jq, jc = meta["jq"], meta["jc"]
    bz = meta.get("bz", False)
    npad = meta["npad"]
    f32, i32, i16 = mybir.dt.float32, mybir.dt.int32, mybir.dt.int16
    bf16 = mybir.dt.bfloat16

    n_call = PAIR * jq * P
    cols_call = n_call // 16
    nfin = nt_core // FIN_B

    nc = bacc.Bacc(
        "TRN2",
        target_bir_lowering=False,
        debug=False,
        num_devices=N_CORES,
        num_swdge_queues=NQUEUES,
    )

    xT_d = nc.dram_tensor("xT", [hid, npad], bf16, kind="ExternalInput").ap()
    W_d = nc.dram_tensor("W", [hid, out_dim], bf16, kind="ExternalInput").ap()
    b_d = nc.dram_tensor("b_bcast", [P, out_dim], f32, kind="ExternalInput").ap()
    dinv_d = nc.dram_tensor("dinv", [P, nt_core], f32, kind="ExternalInput").ap()
    dl_d = nc.dram_tensor(
        "dl", [P, nt_core * jc * 2], bf16, kind="ExternalInput"
    ).ap()
    gcnt_d = nc.dram_tensor(
        "gcnt", [1, (nt_core // PAIR) * NQ], i32, kind="ExternalInput"
    ).ap()
    idx_d = nc.dram_tensor(
        "idx16", [P, (nt_core // PAIR) * NQ * cols_call], i16, kind="ExternalInput"
    ).ap()
    # g rows: node n=(t*128+p) at row p*nt_pad + t (128B rows, all payload).
    # Gather strides must be 256B multiples, so descriptors address row
    # PAIRS (g2 rows of 128 elems): sub-table q=(ph,e) = rows of partitions
    # [64*ph, 64*ph+64) at column half e — int16-addressable.
    g_d = nc.dram_tensor("g", [P * nt_pad, GROW], bf16, kind="Internal").ap()
    out_d = nc.dram_tensor("out", [nt_core * P, out_dim], f32, kind="ExternalOutput").ap()

    gw3 = g_d.rearrange("(p t) d -> p t d", p=P)
    g2 = g_d.rearrange("(r e) d -> r (e d)", e=2)
    # gather payloads must be 256B multiples: each descriptor fetches a row
    # PAIR (even+odd tile); the sub-table's parity picks the payload half at
    # matmul-rhs time, so no extra PE/DVE work
    gq_d = [g2[ts(q // 2, 32 * nt_pad), :] for q in range(NQ)]

    do_a = variant not in ("noop",)
    do_b = variant in ("full", "nogath", "nomm", "noS")
    do_gath = variant in ("full", "nomm", "noS")
    do_smm = variant in ("full", "nogath", "noS")
    do_sbuild = variant in ("full", "nogath")

    from contextlib import nullcontext

    with tile.TileContext(nc) as tc:
        with (
            tc.tile_pool(name="const", bufs=1) as const_pool,
            tc.tile_pool(name="work", bufs=3) as work,
            tc.tile_pool(name="fin", bufs=2) as fin_pool,
            tc.tile_pool(name="gath", bufs=GATH_BUFS) as gath_pool,
            tc.tile_pool(name="smat", bufs=8) as smat_pool,
            tc.tile_pool(name="psumA", bufs=2, space="PSUM") as psumA_pool,
            tc.tile_pool(name="psumB", bufs=4, space="PSUM") as psumB_pool,
            tc.For_i(0, loopn) if loopn else nullcontext(),
        ):
            for rep in range(reps):
                # ---- constants ----
                W_sb = const_pool.tile([hid, out_dim], bf16)
                nc.sync.dma_start(W_sb[:], W_d[:])
                b_sb = const_pool.tile([P, out_dim], f32)
                nc.sync.dma_start(b_sb[:], b_d[:])
                dinv = const_pool.tile([P, nt_core], f32)
                nc.sync.dma_start(dinv[:], dinv_d[:])
                dl_sb = const_pool.tile([P, nt_core * jc * 2], bf16)
                nc.sync.dma_start(dl_sb[:], dl_d[:])
                dl_v = dl_sb[:].rearrange("p (t j e) -> p t j e", j=jc, e=2)
                gcnt_sb = const_pool.tile([1, (nt_core // PAIR) * NQ], i32)
                nc.sync.dma_start(gcnt_sb[:], gcnt_d[:])
                idx_sb = const_pool.tile(
                    [P, (nt_core // PAIR) * NQ * cols_call], i16
                )
                nc.sync.dma_start(idx_sb[:], idx_d[:])

                iota_i = const_pool.tile([P, P], i32)
                nc.gpsimd.iota(iota_i[:], pattern=[[1, P]], base=0, channel_multiplier=0)
                iota_b = const_pool.tile([P, P], bf16)
                nc.vector.tensor_copy(iota_b[:], iota_i[:])
                iota2 = iota_b[:].rearrange("p (q e) -> p q e", e=2)
                cnt_regs = [
                    nc.gpsimd.alloc_register(f"gcnt_reg{rep}_{q}") for q in range(NQ)
                ]

                # ---- pass A: g = (dinv*x) @ W in bf16 for all node tiles ----
                # own rows (this core's tiles t < nt_core) are captured into SBUF
                # on the way through for the pass-B self-loop chunk
                own_sb = const_pool.tile([P, nt_core * out_dim], bf16)
                own_v = own_sb[:].rearrange("p (t d) -> p t d", d=out_dim)
                if not do_a:
                    nc.vector.memset(own_sb[:], 0.25)
                XW = 4  # batches per xT load (fewer SP DMA issues)
                xt2 = None
                for tb in range(nt_pad // BATCH_A if do_a else 0):
                    if tb % XW == 0:
                        xt2 = work.tile([hid, XW * BATCH_A * P], bf16, tag="xT")
                        w = min(XW * BATCH_A * P, npad - tb * BATCH_A * P)
                        nc.sync.dma_start(
                            xt2[:, 0:w], xT_d[:, tb * BATCH_A * P :][:, 0:w]
                        )
                    xt = xt2[:, ts(tb % XW, BATCH_A * P)]
                    hp = psumA_pool.tile([P, BATCH_A * out_dim], f32, tag="psA")
                    for k in range(BATCH_A):
                        nc.tensor.matmul(
                            out=hp[:, ts(k, out_dim)],
                            lhsT=xt[:, ts(k, P)],
                            rhs=W_sb[:],
                            start=True,
                            stop=True,
                        )
                    # g rows are pure payload: one contiguous
                    # 2KB-per-partition burst per store
                    gt = work.tile([P, BATCH_A, GROW], bf16, tag="gA")
                    nc.scalar.activation(
                        gt[:, :, 0:out_dim],
                        hp[:].rearrange("p (k d) -> p k d", d=out_dim),
                        mybir.ActivationFunctionType.Copy,
                    )
                    # alternate the store DGE between Act and SP
                    geng = nc.scalar if tb % 2 == 0 else nc.sync
                    geng.dma_start(
                        gw3[:, ts(tb, BATCH_A), :],
                        gt[:].rearrange("p k d -> p (k d)"),
                    )
                    lo = tb * BATCH_A
                    if lo < nt_core:
                        m = min(BATCH_A, nt_core - lo)
                        nc.vector.tensor_copy(
                            own_v[:, lo : lo + m, :], gt[:, :m, 0:out_dim]
                        )

                # ---- pass B ----
                if not do_b:  # timing probes: emit placeholder output stores
                    zt = const_pool.tile([P, out_dim], f32)
                    nc.vector.memset(zt[:], 0.5)
                    for t in range(nt_core):
                        nc.scalar.dma_start(out_d[ts(t, P), :], zt[:])
                S_const = None
                if do_smm and not do_sbuild:  # "noS" probe: one shared S matrix
                    S_const = const_pool.tile([P, jc * P], bf16)
                    nc.vector.memset(S_const[:], 0.0078125)
                gath_const = None
                if do_b and not do_gath:  # "nomm"/"nogath" probe support
                    gath_const = const_pool.tile([P, NQ * PAIR * jq, GSTEP], bf16)
                    nc.vector.memset(gath_const[:], 0.125)
                op = None
                gath = gath_const
                for t in range(nt_core if do_b else 0):
                    pb, e = divmod(t, PAIR)
                    if do_gath and e == 0:
                        gath = gath_pool.tile(
                            [P, NQ * PAIR * jq, GSTEP], bf16, tag="gath"
                        )
                        if TRUNC and pb < GATH_BUFS:
                            # ensure the pool's physical buffers hold finite
                            # data where truncated gathers leave stale bytes
                            nc.vector.memset(gath[:], 0.0)
                        # sub-tables 2ph and 2ph+1 share in_ap (parity only
                        # picks the rhs half) and their idx columns are
                        # adjacent, so one call covers both: halves the
                        # per-call SWDGE fixed overhead
                        for ph in range(2):
                            nc.gpsimd.dma_gather(
                                out_ap=gath[:, ts(ph, 2 * PAIR * jq), :],
                                in_ap=gq_d[2 * ph],
                                idxs_ap=idx_sb[
                                    :, ts(pb * 2 + ph, 2 * cols_call)
                                ],
                                num_idxs=2 * n_call,
                                num_idxs_reg=2 * n_call,
                                elem_size=GSTEP,
                                elem_step=GSTEP,
                                single_packet=2 * n_call <= 1024,
                                queue_num=(pb * 2 + ph) % NQUEUES,
                            )
                    if do_sbuild:
                        # stride-1 inner pair on every operand -> 2x DVE mode
                        S = smat_pool.tile([P, jc * P], bf16, tag="smat")
                        nc.vector.tensor_tensor(
                            out=S[:].rearrange(
                                "p (j q e) -> p j q e", j=jc, e=2
                            ),
                            in0=dl_v[:, t, :, None, :].to_broadcast(
                                [P, jc, P // 2, 2]
                            ),
                            in1=iota2[:, None, :, :].to_broadcast(
                                [P, jc, P // 2, 2]
                            ),
                            op=mybir.AluOpType.is_equal,
                        )
                    else:
                        S = S_const
                    k = t % FIN_B
                    if do_smm:
                        if k == 0:
                            op = psumB_pool.tile([P, FIN_B, out_dim], f32, tag="psB")
                        for cc in range(jc):
                            if cc < NQ * jq:
                                q, j = divmod(cc, jq)
                                # payload half = sub-table's src tile parity
                                rhs = gath[
                                    :,
                                    q * PAIR * jq + e * jq + j,
                                    ts(q % 2, out_dim),
                                ]
                            else:
                                rhs = own_v[:, t, :]
                            nc.tensor.matmul(
                                out=op[:, k, :],
                                lhsT=S[:, ts(cc, P)],
                                rhs=rhs,
                                start=(cc == 0),
                                stop=(cc == jc - 1),
                            )
                    if k == FIN_B - 1:
                        bb = t // FIN_B
                        if do_smm:
                            fsrc = op[:]
                        else:
                            fsrc = own_v[:, ts(bb, FIN_B), :]
                        osig = fin_pool.tile([P, FIN_B, out_dim], f32, tag="osig")
                        if bz:
                            # b == 0: sigmoid(dinv * psum) as one Act op per
                            # tile with a per-partition scale vector
                            for kk in range(FIN_B):
                                tt = bb * FIN_B + kk
                                nc.scalar.activation(
                                    osig[:, kk, :],
                                    fsrc[:, kk, :]
                                    if do_smm
                                    else own_v[:, tt, :],
                                    mybir.ActivationFunctionType.Sigmoid,
                                    scale=dinv[:, tt : tt + 1],
                                )
                        else:
                            ot = fin_pool.tile([P, FIN_B, out_dim], f32, tag="outt")
                            nc.vector.tensor_tensor(
                                out=ot[:],
                                in0=fsrc,
                                in1=dinv[:, ts(bb, FIN_B), None].to_broadcast(
                                    [P, FIN_B, out_dim]
                                ),
                                op=mybir.AluOpType.mult,
                            )
                            ob = fin_pool.tile([P, FIN_B, out_dim], f32, tag="outb")
                            nc.vector.tensor_tensor(
                                out=ob[:],
                                in0=ot[:],
                                in1=b_sb[:, None, :].to_broadcast(
                                    [P, FIN_B, out_dim]
                                ),
                                op=mybir.AluOpType.add,
                            )
                            nc.scalar.activation(
                                osig[:].rearrange("p k d -> p (k d)"),
                                ob[:].rearrange("p k d -> p (k d)"),
                                mybir.ActivationFunctionType.Sigmoid,
                            )
                        nc.sync.dma_start(
                            out_d[ts(bb, FIN_B * P), :].rearrange("(k p) d -> p k d", p=P),
                            osig[:],
                        )

    nc.compile()
    return nc


def _get_program(meta):
    key = tuple(sorted((k, v) for k, v in meta.items()))
    if key not in _prog_cache:
        _prog_cache[key] = build_program(meta)
    return _prog_cache[key]


def make_in_maps(meta, shared, per_core):
    return [dict(shared, **per_core[c]) for c in range(N_CORES)]


def kernel(x, edge_index, W, b) -> np.ndarray:
    x = np.asarray(x, np.float32)
    edge_index = np.asarray(edge_index)
    W = np.asarray(W, np.float32)
    b = np.asarray(b, np.float32)

    meta, shared, per_core = preprocess(x, edge_index, W, b)
    nc = _get_program(meta)
    in_maps = make_in_maps(meta, shared, per_core)
    res = run_bass_kernel_spmd(nc, in_maps, core_ids=list(range(N_CORES)))
    outs = [res.results[c]["out"] for c in range(N_CORES)]
    full = np.concatenate(outs, axis=0)
    return full[: meta["n_nodes"]]

